# revision 4
# baseline (speedup 1.0000x reference)
"""Exponential smoother: out[b,n] = sum_t w[t] * x[b,t,n], w = softmax-ish
normalized exp(-t/tau) decay weights.

Strategy (8 NeuronCores, pure data parallel over B=64):
  - each core handles 8 batches of x[8, 1000, 4096] f32.
  - w[t] = C*e^(-t/tau) decays so fast that t >= 500 contributes < 2e-11
    of the result -- far below fp32 resolution -- so only t < 500 is
    loaded (halves HBM traffic; kernel is memory-bound).
  - w[125c + p] = w[p] * lam^c with lam = e^(-125/tau): load x[b, :500]
    as [125 partitions, 4 chunks, 4096], scale chunk c by lam^c (ACT),
    accumulate chunks (DVE), then one fp32 matmul per 512 cols with
    lhsT = w[0:125] reduces the partition axis.
"""

import numpy as np

import concourse.bacc as bacc
import concourse.bass as bass
import concourse.mybir as mybir
from concourse.bass_utils import run_bass_kernel_spmd
from concourse.tile import TileContext

B, T, N = 64, 1000, 4096
NCORES = 8
BL = B // NCORES  # batches per core
P = 125  # t-chunk size (partition dim); T = 8 * 125
NCHUNK = 4  # keep t < 500; dropped tail < 2e-11 relative
TAU = 20.0
MM_N = 512  # fp32 matmul free-dim max (one PSUM bank)


def _build() -> bass.Bass:
    nc = bacc.Bacc("TRN2", target_bir_lowering=False, debug=False)
    x = nc.dram_tensor("x", [BL, T, N], mybir.dt.float32, kind="ExternalInput")
    w = nc.dram_tensor("w", [P, 1], mybir.dt.float32, kind="ExternalInput")
    out = nc.dram_tensor("out", [BL, N], mybir.dt.float32, kind="ExternalOutput")
    lam = float(np.exp(-P / TAU))

    with TileContext(nc) as tc:
        with (
            tc.tile_pool(name="io", bufs=2) as io_pool,
            tc.tile_pool(name="wp", bufs=1) as w_pool,
            tc.tile_pool(name="op", bufs=2) as out_pool,
            tc.tile_pool(name="ps", bufs=1, space="PSUM") as psum_pool,
        ):
            w_tile = w_pool.tile([P, 1], mybir.dt.float32)
            nc.sync.dma_start(out=w_tile, in_=w[:, :])
            for b in range(BL):
                xt = io_pool.tile([P, NCHUNK, N], mybir.dt.float32, tag="xt")
                src = x[b, 0 : P * NCHUNK, :].rearrange("(c p) n -> p c n", p=P)
                nc.sync.dma_start(out=xt, in_=src)
                for c in range(1, NCHUNK):
                    nc.scalar.mul(xt[:, c, :], xt[:, c, :], lam**c)
                for c in range(1, NCHUNK):
                    nc.vector.tensor_add(
                        out=xt[:, 0, :], in0=xt[:, 0, :], in1=xt[:, c, :]
                    )
                ps = psum_pool.tile([1, N], mybir.dt.float32, tag="ps")
                for j in range(N // MM_N):
                    nc.tensor.matmul(
                        ps[:, j * MM_N : (j + 1) * MM_N],
                        lhsT=w_tile[:, :],
                        rhs=xt[:, 0, j * MM_N : (j + 1) * MM_N],
                        start=True,
                        stop=True,
                    )
                orow = out_pool.tile([1, N], mybir.dt.float32, tag="orow")
                nc.scalar.copy(orow[:, :], ps[:, :])
                nc.sync.dma_start(out=out[b : b + 1, :], in_=orow[:, :])
    nc.compile()
    return nc


_NC = None


def _get_nc() -> bass.Bass:
    global _NC
    if _NC is None:
        _NC = _build()
    return _NC


def _weights() -> np.ndarray:
    # replicate the reference computation in fp32
    w = np.exp(-np.arange(T, dtype=np.float32) / np.float32(TAU))
    w = w / w.sum(dtype=np.float32)
    return w[:P].reshape(P, 1).astype(np.float32)


def kernel(spike_trains: np.ndarray, _trace: bool = False):
    assert spike_trains.shape == (B, T, N), spike_trains.shape
    x = np.ascontiguousarray(spike_trains, dtype=np.float32)
    w = _weights()
    in_maps = [
        {"x": np.ascontiguousarray(x[i * BL : (i + 1) * BL]), "w": w}
        for i in range(NCORES)
    ]
    res = run_bass_kernel_spmd(
        _get_nc(), in_maps, core_ids=list(range(NCORES)), trace=_trace
    )
    out = np.concatenate([r["out"] for r in res.results], axis=0)
    if _trace:
        return out, res
    return out


# revision 7
# speedup vs baseline: 1.0115x; 1.0115x over previous
"""Exponential smoother: out[b,n] = sum_t w[t] * x[b,t,n], w = softmax-ish
normalized exp(-t/tau) decay weights.

Strategy (8 NeuronCores, pure data parallel over B=64):
  - each core handles 8 batches of x[8, 1000, 4096] f32.
  - w[t] = C*e^(-t/tau) decays so fast that t >= 500 contributes < 2e-11
    of the result -- far below fp32 resolution -- so only t < 500 is
    loaded (halves HBM traffic; kernel is memory-bound).
  - w[125c + p] = w[p] * lam^c with lam = e^(-125/tau): load x[b, :500]
    as [125 partitions, 4 chunks, 4096], scale chunk c by lam^c (ACT),
    accumulate chunks (DVE), then one fp32 matmul per 512 cols with
    lhsT = w[0:125] reduces the partition axis.
"""

import numpy as np

import concourse.bacc as bacc
import concourse.bass as bass
import concourse.mybir as mybir
from concourse.bass_utils import run_bass_kernel_spmd
from concourse.tile import TileContext

B, T, N = 64, 1000, 4096
NCORES = 8
BL = B // NCORES  # batches per core
P = 125  # t-chunk size (partition dim); T = 8 * 125
# keep t < 375: the dropped tail sums to 0.49*e^(-375/20) ~= 3.5e-9 absolute
# (~7e-9 relative) -- below half-ulp of the fp32 result, i.e. invisible next
# to the reassociation noise any fp32 evaluation of the reference carries.
NCHUNK = 3
TAU = 20.0
MM_N = 512  # fp32 matmul free-dim max (one PSUM bank)


NQ = 8  # split elementwise work into n-slices for pipelining


def _build() -> bass.Bass:
    nc = bacc.Bacc("TRN2", target_bir_lowering=False, debug=False)
    x = nc.dram_tensor("x", [BL, T, N], mybir.dt.float32, kind="ExternalInput")
    w = nc.dram_tensor("w", [P, 1], mybir.dt.float32, kind="ExternalInput")
    out = nc.dram_tensor("out", [BL, N], mybir.dt.float32, kind="ExternalOutput")
    lam = float(np.exp(-P / TAU))
    NW = N // NQ  # n-slice width

    with TileContext(nc) as tc:
        with (
            tc.tile_pool(name="io", bufs=3) as io_pool,
            tc.tile_pool(name="wp", bufs=1) as w_pool,
            tc.tile_pool(name="op", bufs=2) as out_pool,
            tc.tile_pool(name="ps", bufs=4, space="PSUM") as psum_pool,
        ):
            w_tile = w_pool.tile([P, 1], mybir.dt.float32)
            nc.sync.dma_start(out=w_tile, in_=w[:, :])
            for b in range(BL):
                xt = io_pool.tile([P, NCHUNK, N], mybir.dt.float32, tag="xt")
                src = x[b, 0 : P * NCHUNK, :].rearrange("(c p) n -> p c n", p=P)
                nc.sync.dma_start(out=xt, in_=src)
                orow = out_pool.tile([1, N], mybir.dt.float32, tag="orow")
                for q in range(NQ):
                    s = slice(q * NW, (q + 1) * NW)
                    # scale chunk c by lam^c (ACT, in place), independent ops
                    for c in range(1, NCHUNK):
                        nc.scalar.mul(xt[:, c, s], xt[:, c, s], lam**c)
                    # binary-tree add on DVE: pairs, then combine into chunk 0
                    srcs = list(range(NCHUNK))
                    while len(srcs) > 1:
                        nxt = []
                        for k in range(0, len(srcs) - 1, 2):
                            a, bb = srcs[k], srcs[k + 1]
                            nc.vector.tensor_add(
                                out=xt[:, a, s], in0=xt[:, a, s], in1=xt[:, bb, s]
                            )
                            nxt.append(a)
                        if len(srcs) % 2:
                            nxt.append(srcs[-1])
                        srcs = nxt
                    # partition-axis reduction with the weight column
                    ps_q = psum_pool.tile([1, NW], mybir.dt.float32, tag="ps")
                    for j in range(NW // MM_N):
                        sj = slice(q * NW + j * MM_N, q * NW + (j + 1) * MM_N)
                        nc.tensor.matmul(
                            ps_q[:, j * MM_N : (j + 1) * MM_N],
                            lhsT=w_tile[:, :],
                            rhs=xt[:, 0, sj],
                            start=True,
                            stop=True,
                        )
                    nc.scalar.copy(orow[:, s], ps_q[:, :])
                nc.sync.dma_start(out=out[b : b + 1, :], in_=orow[:, :])
    nc.compile()
    return nc


_NC = None


def _get_nc() -> bass.Bass:
    global _NC
    if _NC is None:
        _NC = _build()
    return _NC


def _weights() -> np.ndarray:
    # replicate the reference computation in fp32
    w = np.exp(-np.arange(T, dtype=np.float32) / np.float32(TAU))
    w = w / w.sum(dtype=np.float32)
    return w[:P].reshape(P, 1).astype(np.float32)


def kernel(spike_trains: np.ndarray, _trace: bool = False):
    assert spike_trains.shape == (B, T, N), spike_trains.shape
    x = np.ascontiguousarray(spike_trains, dtype=np.float32)
    w = _weights()
    in_maps = [
        {"x": np.ascontiguousarray(x[i * BL : (i + 1) * BL]), "w": w}
        for i in range(NCORES)
    ]
    res = run_bass_kernel_spmd(
        _get_nc(), in_maps, core_ids=list(range(NCORES)), trace=_trace
    )
    out = np.concatenate([r["out"] for r in res.results], axis=0)
    if _trace:
        return out, res
    return out
